# revision 21
# baseline (speedup 1.0000x reference)
"""Trainium2 Bass kernel for a 2-layer shared-weight LSTM with residual.

Problem: x:[1024,200,128], W/U:[128,512], b:[512]; two stacked LSTM layers
sharing (W,U,b); layer 2 has a residual connection; seq_len is ignored by the
reference (full T steps).

Sharding: data-parallel over batch: 1024 = 8 cores x 128 rows.

Device layout ("orientation B"): features/gates on SBUF partitions, batch on
the free axis.  Host pre-transposes x to [T, D, B_local] (bf16) so each
timestep tile is [D=128 partitions, B=128 free] and DMAs straight in.

Fused-unit schedule: unit u (u=0..T) computes layer-2 step u-1 and layer-1
step u together.  For each gate chunk k the PSUM tile holds
    cols 0:128   = z2(u-1) = W_k h1(u-1) + U_k h2raw(u-2) + U_k h1(u-2) + b_k
    cols 128:256 = z1(u)   = W_k x(u)    + U_k h1(u-1)                  + b_k
The layer-2 recurrent input h2n = h2raw + h1 is split across two matmuls so
the residual add is off the recurrence cycle entirely (it only feeds the y
output DMA, on GpSimd).  Matmuls whose inputs are a unit old (U_k h1(u-2),
W_k x(u)) are issued early so only three N=128 matmuls sit between h-ready
and the first gate activation.  Gate activations are merged [128,256]
ScalarE ops (bias fused; per-partition because gates live on partitions);
the c/h pointwise tail is split into L1/L2 halves to shorten the serial
recurrence.  Matmuls run in bf16 (fp32 runs 2-pass LOW_HIGH at half speed);
the c state stays fp32.
"""

import numpy as np
import ml_dtypes

import concourse.bass as bass
import concourse.tile as tile
from concourse import bacc, mybir
from concourse.bass_utils import run_bass_kernel_spmd

B, T, D = 1024, 200, 128
NCORES = 8
BL = B // NCORES  # 128 batch rows per core

F32 = mybir.dt.float32
import os
BF16 = mybir.dt.float32 if os.environ.get("K_FP32") else mybir.dt.bfloat16

# gate order in W/U/b: i, f, g, o  (Keras LSTMCell)
GI, GF, GG, GO = 0, 1, 2, 3
CHUNKS = (GF, GI, GG, GO)  # f first: the c-path needs sig(f) earliest


def _build(nc):
    x_d = nc.dram_tensor("x", [T, D, BL], BF16, kind="ExternalInput")
    w_d = nc.dram_tensor("w", [D, 4 * D], BF16, kind="ExternalInput")
    u_d = nc.dram_tensor("u", [D, 4 * D], BF16, kind="ExternalInput")
    b_d = nc.dram_tensor("bias", [D, 4], F32, kind="ExternalInput")
    y_d = nc.dram_tensor("y", [T, D, BL], BF16, kind="ExternalOutput")

    SIG = mybir.ActivationFunctionType.Sigmoid
    TANH = mybir.ActivationFunctionType.Tanh

    L2 = slice(0, BL)           # layer-2 half (cols 0:128)
    L1 = slice(BL, 2 * BL)      # layer-1 half (cols 128:256)

    with tile.TileContext(nc) as tc:
        with (
            tc.tile_pool(name="singles", bufs=1) as singles,
            tc.tile_pool(name="hbuf", bufs=5) as hpool,
            tc.tile_pool(name="psum", bufs=2, space="PSUM") as pspool,
            tc.tile_pool(name="gates", bufs=2) as gpool,
            tc.tile_pool(name="yst", bufs=3) as ypool,
        ):
            w_sb = singles.tile([D, 4 * D], BF16)
            u_sb = singles.tile([D, 4 * D], BF16)
            b_sb = singles.tile([D, 4], F32)
            nc.sync.dma_start(w_sb[:], w_d[:])
            nc.sync.dma_start(u_sb[:], u_d[:])
            nc.sync.dma_start(b_sb[:], b_d[:])

            # persistent cell state: cols 0:128 = c2, cols 128:256 = c1
            # (fp32: keeps the c trajectory accurate over 200 steps)
            c_both = singles.tile([D, 2 * BL], F32)
            nc.vector.memset(c_both[:], 0.0)

            def wk(k):
                return w_sb[:, k * D:(k + 1) * D]

            def uk(k):
                return u_sb[:, k * D:(k + 1) * D]

            def bk(k):
                return b_sb[:, k:k + 1]

            # ring: hb[u] = [h2raw(u-1) | h1(u) | x(u+1)]   (bf16)
            def new_hb():
                return hpool.tile([D, 3 * BL], BF16, tag="hbuf", name="hbuf")

            def new_ps():
                return {k: pspool.tile([D, 2 * BL], F32, tag=f"ps{k}",
                                       name=f"ps{k}") for k in range(4)}

            def new_gates(names):
                return {n: gpool.tile([D, 2 * BL], BF16, tag=n, name=n)
                        for n in names}

            # hb_pre carries x(0) for unit 0
            hb_pre = new_hb()
            nc.sync.dma_start(hb_pre[:, 2 * BL:3 * BL], x_d[0])

            # ---------------- unit 0: layer-1 step 0 only ----------------
            # z1(0) = W x(0) + b ;  c1(0) = sig(i)*tanh(g) ; h1(0)=sig(o)*tanh(c1)
            hb0 = new_hb()
            nc.sync.dma_start(hb0[:, 2 * BL:3 * BL], x_d[1])
            ps = new_ps()
            for k in CHUNKS:
                nc.tensor.matmul(ps[k][:, L1], wk(k),
                                 hb_pre[:, 2 * BL:3 * BL],
                                 start=True, stop=True)
            g = new_gates(["gf", "gi", "gg", "go", "tc1", "tc2"])
            nc.scalar.activation(g["gi"][:, L1], ps[GI][:, L1], SIG, bias=bk(GI))
            nc.scalar.activation(g["gg"][:, L1], ps[GG][:, L1], TANH, bias=bk(GG))
            nc.scalar.activation(g["go"][:, L1], ps[GO][:, L1], SIG, bias=bk(GO))
            nc.vector.tensor_mul(c_both[:, L1], g["gi"][:, L1], g["gg"][:, L1])
            nc.scalar.activation(g["tc1"][:, L1], c_both[:, L1], TANH)
            nc.vector.tensor_mul(hb0[:, L1], g["go"][:, L1], g["tc1"][:, L1])

            hb = {-1: hb_pre, 0: hb0}
            prev_u2f = None

            # -------- units 1..T-1: fused L2(u-1) + L1(u) --------
            for u in range(1, T):
                hb_u = new_hb()
                hb[u] = hb_u
                if u + 1 < T:
                    nc.sync.dma_start(hb_u[:, 2 * BL:3 * BL], x_d[u + 1])

                ps = new_ps()
                # PSUM bank opener must be the FIRST matmul per bank
                # (start=True resets the bank's has_written).  Inputs that are
                # a unit old run EARLY (before h1(u-1) exists): the opener
                # U_k h1(u-2) and the W_k x(u) half.  Late (after h1/h2raw):
                # W_k h1(u-1), U_k h1(u-1), U_k h2raw(u-2) -- only three short
                # matmuls sit between h-ready and the first gate activation.
                g = new_gates(["gf", "gi", "gg", "go", "tc1", "tc2"])
                m = {n: gpool.tile([D, 2 * BL], F32, tag=n, name=n)
                     for n in ("m1a", "m1b")}
                m.update({n: gpool.tile([D, 2 * BL], BF16, tag=n, name=n)
                          for n in ("m2a", "m2b")})
                FUNC = {GF: SIG, GI: SIG, GG: TANH, GO: SIG}
                NM = {GF: "gf", GI: "gi", GG: "gg", GO: "go"}

                if u >= 2:
                    for k in CHUNKS:
                        mo = nc.tensor.matmul(ps[k][:, L2], uk(k),
                                              hb[u - 2][:, BL:2 * BL],
                                              start=True, stop=False)
                        mx = nc.tensor.matmul(ps[k][:, L1], wk(k),
                                              hb[u - 1][:, 2 * BL:3 * BL],
                                              start=False, stop=False)
                        if prev_u2f is not None:
                            # keep next-unit early matmuls out of the
                            # critical f-chunk window of THIS unit
                            for mm in (mo, mx):
                                tile.add_dep_helper(
                                    mm.ins, prev_u2f.ins, sync=False,
                                    reason="early after prev U_f(h2raw)")
                    mm_w = {}
                    mm_u2 = {}
                    for k in CHUNKS:
                        mm_w[k] = nc.tensor.matmul(
                            ps[k][:, L2], wk(k), hb[u - 1][:, BL:2 * BL],
                            start=False, stop=False)
                        nc.tensor.matmul(ps[k][:, L1], uk(k),
                                         hb[u - 1][:, BL:2 * BL],
                                         start=False, stop=True)
                        mm_u2[k] = nc.tensor.matmul(
                            ps[k][:, L2], uk(k), hb[u - 1][:, 0:BL],
                            start=False, stop=True)
                        # gate ACT right after its chunk's last matmul keeps
                        # the engine-count wait threshold tight
                        nc.scalar.activation(g[NM[k]][:], ps[k][:],
                                             FUNC[k], bias=bk(k))
                    # the scheduler otherwise sinks the h2raw matmuls to
                    # the end, delaying the gate-activation ladder
                    for ka, kb in ((GI, GF), (GG, GI), (GO, GG)):
                        tile.add_dep_helper(
                            mm_w[ka].ins, mm_u2[kb].ins, sync=False,
                            reason="chunk ladder order")
                    prev_u2f = mm_u2[GF]
                else:
                    for k in CHUNKS:
                        nc.tensor.matmul(ps[k][:, 0:2 * BL], wk(k),
                                         hb[u - 1][:, BL:3 * BL],
                                         start=True, stop=False)
                        nc.tensor.matmul(ps[k][:, L1], uk(k),
                                         hb[u - 1][:, BL:2 * BL],
                                         start=False, stop=True)
                        nc.scalar.activation(g[NM[k]][:], ps[k][:],
                                             FUNC[k], bias=bk(k))

                # L1 tail (critical recurrence): c1' -> tanh -> h1(u)
                nc.vector.tensor_mul(m["m1a"][:, L1], g["gf"][:, L1],
                                     c_both[:, L1])
                nc.vector.tensor_mul(m["m2a"][:, L1], g["gi"][:, L1],
                                     g["gg"][:, L1])
                nc.vector.tensor_add(c_both[:, L1], m["m1a"][:, L1],
                                     m["m2a"][:, L1])
                nc.scalar.activation(g["tc1"][:, L1], c_both[:, L1], TANH)
                # L2 tail: c2' -> tanh -> h2raw(u-1)
                nc.vector.tensor_mul(m["m1b"][:, L2], g["gf"][:, L2],
                                     c_both[:, L2])
                nc.vector.tensor_mul(m["m2b"][:, L2], g["gi"][:, L2],
                                     g["gg"][:, L2])
                nc.vector.tensor_mul(hb_u[:, L1], g["go"][:, L1],
                                     g["tc1"][:, L1])          # h1(u)
                nc.vector.tensor_add(c_both[:, L2], m["m1b"][:, L2],
                                     m["m2b"][:, L2])
                nc.scalar.activation(g["tc2"][:, L2], c_both[:, L2], TANH)
                nc.vector.tensor_mul(hb_u[:, L2], g["go"][:, L2],
                                     g["tc2"][:, L2])          # h2raw(u-1)

                # residual -> y(u-1): off the recurrence, on GpSimd
                yt = ypool.tile([D, BL], BF16, tag="yst", name="yst")
                nc.gpsimd.tensor_add(yt[:], hb_u[:, 0:BL],
                                     hb[u - 1][:, BL:2 * BL])
                nc.sync.dma_start(y_d[u - 1], yt[:])

                hb.pop(u - 3, None)

            # ---------------- unit T: layer-2 step T-1 only ----------------
            u = T
            ps = new_ps()
            for k in CHUNKS:
                nc.tensor.matmul(ps[k][:, L2], uk(k), hb[u - 2][:, BL:2 * BL],
                                 start=True, stop=False)
                nc.tensor.matmul(ps[k][:, L2], wk(k), hb[u - 1][:, BL:2 * BL],
                                 start=False, stop=False)
                nc.tensor.matmul(ps[k][:, L2], uk(k), hb[u - 1][:, 0:BL],
                                 start=False, stop=True)
            g = new_gates(["gf", "gi", "gg", "go", "tc2"])
            m = {"m1b": gpool.tile([D, 2 * BL], F32, tag="m1b", name="m1b"),
                 "m2b": gpool.tile([D, 2 * BL], BF16, tag="m2b", name="m2b")}
            nc.scalar.activation(g["gf"][:, L2], ps[GF][:, L2], SIG, bias=bk(GF))
            nc.scalar.activation(g["gi"][:, L2], ps[GI][:, L2], SIG, bias=bk(GI))
            nc.scalar.activation(g["gg"][:, L2], ps[GG][:, L2], TANH, bias=bk(GG))
            nc.scalar.activation(g["go"][:, L2], ps[GO][:, L2], SIG, bias=bk(GO))
            nc.vector.tensor_mul(m["m1b"][:, L2], g["gf"][:, L2], c_both[:, L2])
            nc.vector.tensor_mul(m["m2b"][:, L2], g["gi"][:, L2], g["gg"][:, L2])
            nc.vector.tensor_add(c_both[:, L2], m["m1b"][:, L2], m["m2b"][:, L2])
            nc.scalar.activation(g["tc2"][:, L2], c_both[:, L2], TANH)
            hraw = ypool.tile([D, BL], BF16, tag="yst", name="hraw")
            nc.vector.tensor_mul(hraw[:], g["go"][:, L2], g["tc2"][:, L2])
            yt = ypool.tile([D, BL], BF16, tag="yst", name="yst")
            nc.gpsimd.tensor_add(yt[:], hraw[:], hb[u - 1][:, BL:2 * BL])
            nc.sync.dma_start(y_d[T - 1], yt[:])

    nc.finalize()
    return nc


_CACHED = {}


def _get_nc():
    if "nc" not in _CACHED:
        nc = bacc.Bacc("TRN2", target_bir_lowering=False, debug=False,
                       num_devices=NCORES)
        _CACHED["nc"] = _build(nc)
    return _CACHED["nc"]


def kernel(x, W, U, b, seq_len):
    assert x.shape == (B, T, D)
    nc = _get_nc()

    import os
    bf = np.float32 if os.environ.get("K_FP32") else ml_dtypes.bfloat16
    Wc = np.ascontiguousarray(np.asarray(W, dtype=np.float32).astype(bf))
    Uc = np.ascontiguousarray(np.asarray(U, dtype=np.float32).astype(bf))
    bc = np.ascontiguousarray(
        np.asarray(b, dtype=np.float32).reshape(4, D).T)  # [D, 4]

    in_maps = []
    for c in range(NCORES):
        xc = np.ascontiguousarray(
            np.asarray(x[c * BL:(c + 1) * BL], dtype=np.float32)
            .transpose(1, 2, 0).astype(bf))  # [T, D, BL] bf16
        in_maps.append({"x": xc, "w": Wc, "u": Uc, "bias": bc})

    res = run_bass_kernel_spmd(nc, in_maps, core_ids=list(range(NCORES)))

    y = np.empty((B, T, D), dtype=np.float32)
    for c in range(NCORES):
        # y_T [T, D, BL] bf16 -> [BL, T, D] fp32
        y[c * BL:(c + 1) * BL] = (
            res.results[c]["y"].astype(np.float32).transpose(2, 0, 1))
    return y
